# revision 1
# baseline (speedup 1.0000x reference)
"""Trainium2 Bass kernel for a binarized BasicBlock (2x bconv3x3 + BN +
residual hardtanh + channel shuffle), data-parallel over batch on 8 cores.

Self-contained: hardcodes shapes from the problem spec.
  x: (32, 256, 56, 56) f32 -> out: (32, 256, 56, 56) f32
"""

import numpy as np
import ml_dtypes

import concourse.bass as bass
import concourse.tile as tile
from concourse import bacc, mybir
from concourse import bass_utils

EPS = 1e-5
P = 128
H = W = 56
WP = 58          # padded row width
RP = 59          # padded rows allocated (58 used + 1 spare for tail matmul reads)
NFLAT = RP * WP  # flat free size of padded buffers
IMGS_PER_CORE = 4
NCORES = 8
NT = 7           # row tiles per image (8 output rows each)
TF = 8 * WP      # matmul free size per tile = 464

F32 = mybir.dt.float32
BF16 = mybir.dt.bfloat16
ALU = mybir.AluOpType
ACTF = mybir.ActivationFunctionType

_CACHE = {}


def _flat(ap3):
    return ap3.rearrange("p r c -> p (r c)")


def _build():
    nc = bacc.Bacc("TRN2", target_bir_lowering=False, debug=False)

    x_h = nc.dram_tensor("xs", [IMGS_PER_CORE, 2 * P, H, W], F32, kind="ExternalInput")
    w1_h = nc.dram_tensor("w1m", [P, 9 * P], BF16, kind="ExternalInput")
    w2_h = nc.dram_tensor("w2m", [P, 9 * P], BF16, kind="ExternalInput")
    cst_h = nc.dram_tensor("cst", [P, 16], F32, kind="ExternalInput")
    out_h = nc.dram_tensor("out", [IMGS_PER_CORE, 2 * P, H, W], F32, kind="ExternalOutput")

    x_ap = x_h.ap()

    def out_ch4(n, base_ch, nch=64):
        # DRAM AP: channels base_ch, base_ch+4, ... of image n, full spatial.
        return bass.AP(
            tensor=out_h,
            offset=(n * 2 * P + base_ch) * H * W,
            ap=[[4 * H * W, nch], [1, H * W]],
        )

    with tile.TileContext(nc) as tc:
        # persistent ping-pong buffers
        XA = [nc.alloc_sbuf_tensor(f"XA{i}", [P, RP, WP], F32).ap() for i in range(2)]
        A2 = [nc.alloc_sbuf_tensor(f"A2{i}", [P, RP, WP], F32).ap() for i in range(2)]
        B1 = nc.alloc_sbuf_tensor("B1", [P, RP, WP], BF16).ap()
        B2 = nc.alloc_sbuf_tensor("B2", [P, RP, WP], BF16).ap()
        XIH = [nc.alloc_sbuf_tensor(f"XIH{i}", [P, H, W], F32).ap() for i in range(2)]
        FO1 = [nc.alloc_sbuf_tensor(f"FO1{i}", [P, H, W], F32).ap() for i in range(2)]
        OT2 = [nc.alloc_sbuf_tensor(f"OT2{i}", [P, H, W], F32).ap() for i in range(2)]
        WS1 = nc.alloc_sbuf_tensor("WS1", [P, 9 * P], BF16).ap()
        WS2 = nc.alloc_sbuf_tensor("WS2", [P, 9 * P], BF16).ap()
        CST = nc.alloc_sbuf_tensor("CST", [P, 16], F32).ap()

        s1 = CST[:, 0:1]
        b1 = CST[:, 1:2]
        s2 = CST[:, 2:3]
        b2 = CST[:, 3:4]
        beta_hi = CST[64:128, 4:5]
        beta_min = CST[64:128, 5:6]
        beta_max = CST[64:128, 6:7]
        mv0_lo = CST[64:128, 7:8]
        cxh = CST[64:128, 8:9]

        nc.sync.dma_start(out=WS1, in_=w1_h.ap())
        nc.sync.dma_start(out=WS2, in_=w2_h.ap())
        nc.sync.dma_start(out=CST, in_=cst_h.ap())

        # init pads: B1 u-domain pad = 0.5, B2 sign-domain pad = 0.0.
        # Interiors are rewritten per image; pads never touched again.
        nc.gpsimd.memset(_flat(B1), 0.5)
        nc.gpsimd.memset(_flat(B2), 0.0)
        for i in range(2):
            nc.gpsimd.memset(_flat(XA[i]), 0.0)
            nc.gpsimd.memset(_flat(A2[i]), 0.0)

        with (
            tc.tile_pool(name="psum", bufs=4, space="PSUM") as psum_pool,
            tc.tile_pool(name="stage", bufs=4) as stage_pool,
        ):
            for n in range(IMGS_PER_CORE):
                s = n % 2
                xa, a2, xih, fo1, ot2 = XA[s], A2[s], XIH[s], FO1[s], OT2[s]
                xaf, a2f, b1f, b2f = _flat(xa), _flat(a2), _flat(B1), _flat(B2)

                # loads
                nc.sync.dma_start(out=xa[:, 1:57, 1:57], in_=x_ap[n, 0:P])
                nc.sync.dma_start(out=a2[64:128, 1:57, 1:57], in_=x_ap[n, P:P + 64])
                nc.sync.dma_start(out=xih[64:128], in_=x_ap[n, P + 64:2 * P])

                # u1 = (x_act >= 0) in {0,1}; pads stay 0.5
                nc.vector.tensor_scalar(
                    out=B1[:, 1:57, 1:57], in0=xa[:, 1:57, 1:57],
                    scalar1=0.0, scalar2=None, op0=ALU.is_ge)

                # idle halves: bias adds
                nc.gpsimd.tensor_scalar(
                    out=a2[64:128, 1:57, 1:57], in0=a2[64:128, 1:57, 1:57],
                    scalar1=mv0_lo, scalar2=None, op0=ALU.add)
                nc.vector.tensor_scalar(
                    out=xih[64:128], in0=xih[64:128],
                    scalar1=cxh, scalar2=None, op0=ALU.add)
                nc.sync.dma_start(out=out_ch4(n, 3), in_=_flat(xih)[64:128])

                # conv1: out1 = clip(2*inv1*psum_u + b1' + res, -1, 1)
                for t in range(NT):
                    ps = psum_pool.tile([P, TF], F32)
                    for k in range(9):
                        ky, kx = divmod(k, 3)
                        off = (8 * t + ky) * WP + kx
                        nc.tensor.matmul(
                            ps, lhsT=WS1[:, P * k:P * (k + 1)],
                            rhs=b1f[:, off:off + TF],
                            start=(k == 0), stop=(k == 8))
                    t1 = stage_pool.tile([P, TF], F32, tag="t1")
                    nc.scalar.activation(t1, ps, ACTF.Identity, bias=b1, scale=s1)
                    roff = (8 * t + 1) * WP + 1
                    nc.vector.tensor_tensor(
                        out=t1, in0=t1, in1=xaf[:, roff:roff + TF], op=ALU.add)
                    t13 = t1.rearrange("p (r c) -> p r c", c=WP)[:, :, 0:56]
                    # lo channels -> A2 interior (conv2 input/residual)
                    nc.gpsimd.tensor_scalar(
                        out=a2[0:64, 1 + 8 * t:9 + 8 * t, 1:57], in0=t13[0:64],
                        scalar1=1.0, scalar2=-1.0, op0=ALU.min, op1=ALU.max)
                    # hi channels -> clip then +move1_even -> FO1
                    nc.vector.tensor_scalar(
                        out=t1[64:128], in0=t1[64:128],
                        scalar1=beta_hi, scalar2=beta_min, op0=ALU.add, op1=ALU.min)
                    nc.gpsimd.tensor_scalar(
                        out=fo1[64:128, 8 * t:8 * t + 8, :], in0=t13[64:128],
                        scalar1=beta_max, scalar2=None, op0=ALU.max)
                nc.sync.dma_start(out=out_ch4(n, 1), in_=_flat(fo1)[64:128])

                # b2 = sign(a2) in {-1, +1}; pads stay 0.0
                nc.scalar.sign(B2[:, 1:57, 1:57], a2[:, 1:57, 1:57])

                # conv2: out2 = clip(inv2*psum_s + b2' + res2, -1, 1)
                for t in range(NT):
                    ps = psum_pool.tile([P, TF], F32)
                    for k in range(9):
                        ky, kx = divmod(k, 3)
                        off = (8 * t + ky) * WP + kx
                        nc.tensor.matmul(
                            ps, lhsT=WS2[:, P * k:P * (k + 1)],
                            rhs=b2f[:, off:off + TF],
                            start=(k == 0), stop=(k == 8))
                    t2 = stage_pool.tile([P, TF], F32, tag="t2")
                    nc.scalar.activation(t2, ps, ACTF.Identity, bias=b2, scale=s2)
                    roff = (8 * t + 1) * WP + 1
                    nc.vector.tensor_tensor(
                        out=t2, in0=t2, in1=a2f[:, roff:roff + TF], op=ALU.add)
                    t23 = t2.rearrange("p (r c) -> p r c", c=WP)[:, :, 0:56]
                    nc.gpsimd.tensor_scalar(
                        out=ot2[:, 8 * t:8 * t + 8, :], in0=t23,
                        scalar1=1.0, scalar2=-1.0, op0=ALU.min, op1=ALU.max)
                nc.sync.dma_start(out=out_ch4(n, 0), in_=_flat(ot2)[0:64])
                nc.sync.dma_start(out=out_ch4(n, 2), in_=_flat(ot2)[64:128])

    nc.compile()
    return nc


def _host_prep(w1, w2, bn1_gamma, bn1_beta, bn1_mean, bn1_var,
               bn2_gamma, bn2_beta, bn2_mean, bn2_var, move0_bias, move1_bias):
    f8 = np.float64
    bw1 = np.where(w1 >= 0, 1.0, -1.0).astype(f8)   # [co, ci, 3, 3]
    bw2 = np.where(w2 >= 0, 1.0, -1.0).astype(f8)

    # conv1 lhsT layout [ci, 9*co]: col k*128+co = bw1[co, ci, ky, kx]
    w1m = np.ascontiguousarray(
        bw1.transpose(1, 2, 3, 0).reshape(P, 9 * P)).astype(ml_dtypes.bfloat16)

    # conv2 channel permutation (both in and out sides)
    pidx = np.arange(P)
    chan = np.where(pidx < 64, 2 * pidx, 2 * (pidx - 64) + 1)  # partition -> x_act2 channel
    bw2p = bw2[np.ix_(chan, chan)]                  # [co', ci', 3, 3]
    w2m = np.ascontiguousarray(
        bw2p.transpose(1, 2, 3, 0).reshape(P, 9 * P)).astype(ml_dtypes.bfloat16)

    inv1 = bn1_gamma.astype(f8) / np.sqrt(bn1_var.astype(f8) + EPS)
    c0_1 = bw1.sum(axis=(1, 2, 3))
    s1 = 2.0 * inv1
    b1 = bn1_beta.astype(f8) - bn1_mean.astype(f8) * inv1 - inv1 * c0_1

    inv2 = (bn2_gamma.astype(f8) / np.sqrt(bn2_var.astype(f8) + EPS))[chan]
    s2 = inv2
    b2 = bn2_beta.astype(f8)[chan] - bn2_mean.astype(f8)[chan] * inv2

    cst = np.zeros((P, 16), np.float64)
    cst[:, 0] = s1
    cst[:, 1] = b1
    cst[:, 2] = s2
    cst[:, 3] = b2
    i = np.arange(64)
    cst[64:128, 4] = move1_bias[2 * i]
    cst[64:128, 5] = 1.0 + move1_bias[2 * i]
    cst[64:128, 6] = -1.0 + move1_bias[2 * i]
    cst[64:128, 7] = move0_bias[i]
    cst[64:128, 8] = move0_bias[64 + i] + move1_bias[2 * i + 1]
    return w1m, w2m, cst.astype(np.float32)


def kernel(x, w1, w2, bn1_gamma, bn1_beta, bn1_mean, bn1_var,
           bn2_gamma, bn2_beta, bn2_mean, bn2_var, move0_bias, move1_bias,
           _trace=False):
    x = np.asarray(x, np.float32)
    args = [np.asarray(a, np.float32) for a in (
        w1, w2, bn1_gamma, bn1_beta, bn1_mean, bn1_var,
        bn2_gamma, bn2_beta, bn2_mean, bn2_var, move0_bias, move1_bias)]
    w1m, w2m, cst = _host_prep(*args)

    if "nc" not in _CACHE:
        _CACHE["nc"] = _build()
    nc = _CACHE["nc"]

    in_maps = [
        {"xs": np.ascontiguousarray(x[IMGS_PER_CORE * c:IMGS_PER_CORE * (c + 1)]),
         "w1m": w1m, "w2m": w2m, "cst": cst}
        for c in range(NCORES)
    ]
    kw = {}
    if _trace:
        kw = dict(trace=True, trace_kwargs={"title": "basicblock"})
    res = bass_utils.run_bass_kernel_spmd(nc, in_maps, core_ids=list(range(NCORES)), **kw)
    out = np.concatenate([res.results[c]["out"] for c in range(NCORES)], axis=0)
    if _trace:
        _CACHE["last_results"] = res
    return out


# revision 2
# speedup vs baseline: 2.9187x; 2.9187x over previous
"""Trainium2 Bass kernel for a binarized BasicBlock (2x bconv3x3 + BN +
residual hardtanh + channel shuffle), data-parallel over batch on 8 cores.

Self-contained: hardcodes shapes from the problem spec.
  x: (32, 256, 56, 56) f32 -> out: (32, 256, 56, 56) f32

Layout strategy:
- activations/residuals kept compact [128, 56, 56] in SBUF (efficient DMA)
- only the binarized conv operands live in a zero/half-padded 58-wide
  layout [128, 59, 58]; the binarize op itself (is_ge -> {0,1} "u-domain",
  pads 0.5, correction folded into BN bias on host) does the spread.
- conv = 9 accumulating matmuls (one per tap) into PSUM [128, 464] per
  8-row tile; junk at cols 56/57 of each row is never consumed.
- both channel_shuffles are free: host-permuted w2 channels + stride-4
  channel DMA writes.
"""

import numpy as np
import ml_dtypes

import concourse.bass as bass
import concourse.tile as tile
from concourse import bacc, mybir
from concourse import bass_utils

EPS = 1e-5
P = 128
H = W = 56
WP = 58          # padded row width
RP = 59          # padded rows allocated (58 used + 1 spare for tail matmul reads)
IMGS_PER_CORE = 4
NCORES = 8
NT = 7           # row tiles per image (8 output rows each)
TF = 8 * WP      # matmul free size per tile = 464
CF = 8 * W       # compact free size per tile = 448

F32 = mybir.dt.float32
BF16 = mybir.dt.bfloat16
ALU = mybir.AluOpType
ACTF = mybir.ActivationFunctionType

_CACHE = {}


def _flat(ap3):
    return ap3.rearrange("p r c -> p (r c)")


def _build():
    nc = bacc.Bacc("TRN2", target_bir_lowering=False, debug=False)

    x_h = nc.dram_tensor("xs", [IMGS_PER_CORE, 2 * P, H, W], F32, kind="ExternalInput")
    w1_h = nc.dram_tensor("w1m", [P, 9 * P], BF16, kind="ExternalInput")
    w2_h = nc.dram_tensor("w2m", [P, 9 * P], BF16, kind="ExternalInput")
    cst_h = nc.dram_tensor("cst", [P, 16], F32, kind="ExternalInput")
    out_h = nc.dram_tensor("out", [IMGS_PER_CORE, 2 * P, H, W], F32, kind="ExternalOutput")

    x_ap = x_h.ap()

    def out_ch4(n, base_ch, nch=64):
        # DRAM AP: channels base_ch, base_ch+4, ... of image n, full spatial.
        return bass.AP(
            tensor=out_h,
            offset=(n * 2 * P + base_ch) * H * W,
            ap=[[4 * H * W, nch], [1, H * W]],
        )

    with tile.TileContext(nc) as tc:
        # persistent ping-pong buffers (compact except B1/B2)
        XA = [nc.alloc_sbuf_tensor(f"XA{i}", [P, H, W], F32).ap() for i in range(2)]
        A2 = [nc.alloc_sbuf_tensor(f"A2{i}", [P, H, W], F32).ap() for i in range(2)]
        B1 = nc.alloc_sbuf_tensor("B1", [P, RP, WP], BF16).ap()
        B2 = nc.alloc_sbuf_tensor("B2", [P, RP, WP], BF16).ap()
        XIH = [nc.alloc_sbuf_tensor(f"XIH{i}", [P, H, W], F32).ap() for i in range(2)]
        FO1 = [nc.alloc_sbuf_tensor(f"FO1{i}", [P, H, W], F32).ap() for i in range(2)]
        OT2 = [nc.alloc_sbuf_tensor(f"OT2{i}", [P, H, W], F32).ap() for i in range(2)]
        WS1 = nc.alloc_sbuf_tensor("WS1", [P, 9 * P], BF16).ap()
        WS2 = nc.alloc_sbuf_tensor("WS2", [P, 9 * P], BF16).ap()
        CST = nc.alloc_sbuf_tensor("CST", [P, 16], F32).ap()

        s1 = CST[:, 0:1]
        b1 = CST[:, 1:2]
        s2 = CST[:, 2:3]
        b2 = CST[:, 3:4]
        beta_hi = CST[64:128, 4:5]
        mv0_lo = CST[64:128, 7:8]
        cxh = CST[64:128, 8:9]

        nc.sync.dma_start(out=WS1, in_=w1_h.ap())
        nc.sync.dma_start(out=WS2, in_=w2_h.ap())
        nc.sync.dma_start(out=CST, in_=cst_h.ap())

        # u-domain pads: 0.5 stands for binarized zero-padding. Interiors
        # are rewritten per image; pads never touched again.
        nc.gpsimd.memset(_flat(B1), 0.5)
        nc.gpsimd.memset(_flat(B2), 0.5)

        with (
            tc.tile_pool(name="psum", bufs=6, space="PSUM") as psum_pool,
            tc.tile_pool(name="stage", bufs=6) as stage_pool,
        ):
            for n in range(IMGS_PER_CORE):
                s = n % 2
                xa, a2, xih, fo1, ot2 = XA[s], A2[s], XIH[s], FO1[s], OT2[s]
                xaf, a2f, b1f, b2f = _flat(xa), _flat(a2), _flat(B1), _flat(B2)

                # loads (all-compact destinations -> large DMA descriptors)
                nc.sync.dma_start(out=xa, in_=x_ap[n, 0:P])
                nc.sync.dma_start(out=a2[64:128], in_=x_ap[n, P:P + 64])
                nc.sync.dma_start(out=xih[64:128], in_=x_ap[n, P + 64:2 * P])

                # u1 = (x_act >= 0) in {0,1}, spread into padded B1
                nc.vector.tensor_scalar(
                    out=B1[:, 1:57, 1:57], in0=xa,
                    scalar1=0.0, scalar2=None, op0=ALU.is_ge)

                # idle halves: bias adds (AP scalars -> ACT/DVE only)
                nc.scalar.activation(
                    a2[64:128], a2[64:128], ACTF.Identity, bias=mv0_lo)
                nc.scalar.activation(
                    xih[64:128], xih[64:128], ACTF.Identity, bias=cxh)
                nc.sync.dma_start(out=out_ch4(n, 3), in_=_flat(xih)[64:128])

                # conv1: out1 = clip(2*inv1*psum_u + b1' + res, -1, 1)
                for t in range(NT):
                    ps = psum_pool.tile([P, TF], F32)
                    for k in range(9):
                        ky, kx = divmod(k, 3)
                        off = (8 * t + ky) * WP + kx
                        nc.tensor.matmul(
                            ps, lhsT=WS1[:, P * k:P * (k + 1)],
                            rhs=b1f[:, off:off + TF],
                            start=(k == 0), stop=(k == 8))
                    ps3 = ps.rearrange("p (r c) -> p r c", c=WP)[:, :, 0:W]
                    t1 = stage_pool.tile([P, CF], F32, tag="t1")
                    nc.scalar.activation(t1, ps3, ACTF.Identity, bias=b1, scale=s1)
                    nc.vector.tensor_tensor(
                        out=t1, in0=t1, in1=xaf[:, CF * t:CF * (t + 1)], op=ALU.add)
                    # lo channels -> A2 (conv2 input/residual)
                    nc.gpsimd.tensor_scalar(
                        out=a2[0:64, 8 * t:8 * t + 8, :],
                        in0=t1.rearrange("p (r c) -> p r c", c=W)[0:64],
                        scalar1=1.0, scalar2=-1.0, op0=ALU.min, op1=ALU.max)
                    # hi channels: clip in place, then +move1_even -> FO1
                    nc.gpsimd.tensor_scalar(
                        out=t1[64:128], in0=t1[64:128],
                        scalar1=1.0, scalar2=-1.0, op0=ALU.min, op1=ALU.max)
                    nc.vector.tensor_scalar(
                        out=_flat(fo1)[64:128, CF * t:CF * (t + 1)], in0=t1[64:128],
                        scalar1=beta_hi, scalar2=None, op0=ALU.add)
                nc.sync.dma_start(out=out_ch4(n, 1), in_=_flat(fo1)[64:128])

                # u2 = (a2 >= 0) in {0,1}, spread into padded B2
                nc.vector.tensor_scalar(
                    out=B2[:, 1:57, 1:57], in0=a2,
                    scalar1=0.0, scalar2=None, op0=ALU.is_ge)

                # conv2: out2 = clip(2*inv2*psum_u + b2' + res2, -1, 1)
                for t in range(NT):
                    ps = psum_pool.tile([P, TF], F32)
                    for k in range(9):
                        ky, kx = divmod(k, 3)
                        off = (8 * t + ky) * WP + kx
                        nc.tensor.matmul(
                            ps, lhsT=WS2[:, P * k:P * (k + 1)],
                            rhs=b2f[:, off:off + TF],
                            start=(k == 0), stop=(k == 8))
                    ps3 = ps.rearrange("p (r c) -> p r c", c=WP)[:, :, 0:W]
                    t2 = stage_pool.tile([P, CF], F32, tag="t2")
                    nc.scalar.activation(t2, ps3, ACTF.Identity, bias=b2, scale=s2)
                    nc.vector.tensor_tensor(
                        out=t2, in0=t2, in1=a2f[:, CF * t:CF * (t + 1)], op=ALU.add)
                    nc.gpsimd.tensor_scalar(
                        out=ot2[:, 8 * t:8 * t + 8, :],
                        in0=t2.rearrange("p (r c) -> p r c", c=W),
                        scalar1=1.0, scalar2=-1.0, op0=ALU.min, op1=ALU.max)
                nc.sync.dma_start(out=out_ch4(n, 0), in_=_flat(ot2)[0:64])
                nc.sync.dma_start(out=out_ch4(n, 2), in_=_flat(ot2)[64:128])

    nc.compile()
    return nc


def _host_prep(w1, w2, bn1_gamma, bn1_beta, bn1_mean, bn1_var,
               bn2_gamma, bn2_beta, bn2_mean, bn2_var, move0_bias, move1_bias):
    f8 = np.float64
    bw1 = np.where(w1 >= 0, 1.0, -1.0).astype(f8)   # [co, ci, 3, 3]
    bw2 = np.where(w2 >= 0, 1.0, -1.0).astype(f8)

    # conv1 lhsT layout [ci, 9*co]: col k*128+co = bw1[co, ci, ky, kx]
    w1m = np.ascontiguousarray(
        bw1.transpose(1, 2, 3, 0).reshape(P, 9 * P)).astype(ml_dtypes.bfloat16)

    # conv2 channel permutation (both in and out sides)
    pidx = np.arange(P)
    chan = np.where(pidx < 64, 2 * pidx, 2 * (pidx - 64) + 1)  # partition -> x_act2 channel
    bw2p = bw2[np.ix_(chan, chan)]                  # [co', ci', 3, 3]
    w2m = np.ascontiguousarray(
        bw2p.transpose(1, 2, 3, 0).reshape(P, 9 * P)).astype(ml_dtypes.bfloat16)

    # u-domain: conv_sign = 2*conv_u - c0, c0 = sum of signed weights
    inv1 = bn1_gamma.astype(f8) / np.sqrt(bn1_var.astype(f8) + EPS)
    c0_1 = bw1.sum(axis=(1, 2, 3))
    s1 = 2.0 * inv1
    b1 = bn1_beta.astype(f8) - bn1_mean.astype(f8) * inv1 - inv1 * c0_1

    inv2 = (bn2_gamma.astype(f8) / np.sqrt(bn2_var.astype(f8) + EPS))[chan]
    c0_2 = bw2.sum(axis=(1, 2, 3))[chan]
    s2 = 2.0 * inv2
    b2 = bn2_beta.astype(f8)[chan] - bn2_mean.astype(f8)[chan] * inv2 - inv2 * c0_2

    cst = np.zeros((P, 16), np.float64)
    cst[:, 0] = s1
    cst[:, 1] = b1
    cst[:, 2] = s2
    cst[:, 3] = b2
    i = np.arange(64)
    cst[64:128, 4] = move1_bias[2 * i]
    cst[64:128, 7] = move0_bias[i]
    cst[64:128, 8] = move0_bias[64 + i] + move1_bias[2 * i + 1]
    return w1m, w2m, cst.astype(np.float32)


def kernel(x, w1, w2, bn1_gamma, bn1_beta, bn1_mean, bn1_var,
           bn2_gamma, bn2_beta, bn2_mean, bn2_var, move0_bias, move1_bias,
           _trace=False):
    x = np.asarray(x, np.float32)
    args = [np.asarray(a, np.float32) for a in (
        w1, w2, bn1_gamma, bn1_beta, bn1_mean, bn1_var,
        bn2_gamma, bn2_beta, bn2_mean, bn2_var, move0_bias, move1_bias)]
    w1m, w2m, cst = _host_prep(*args)

    if "nc" not in _CACHE:
        _CACHE["nc"] = _build()
    nc = _CACHE["nc"]

    in_maps = [
        {"xs": np.ascontiguousarray(x[IMGS_PER_CORE * c:IMGS_PER_CORE * (c + 1)]),
         "w1m": w1m, "w2m": w2m, "cst": cst}
        for c in range(NCORES)
    ]
    kw = {}
    if _trace:
        kw = dict(trace=True, trace_kwargs={"title": "basicblock"})
    res = bass_utils.run_bass_kernel_spmd(nc, in_maps, core_ids=list(range(NCORES)), **kw)
    out = np.concatenate([res.results[c]["out"] for c in range(NCORES)], axis=0)
    if _trace:
        _CACHE["last_results"] = res
    return out


# revision 4
# speedup vs baseline: 3.0766x; 1.0541x over previous
"""Trainium2 Bass kernel for a binarized BasicBlock (2x bconv3x3 + BN +
residual hardtanh + channel shuffle), data-parallel over batch on 8 cores.

Self-contained: hardcodes shapes from the problem spec.
  x: (32, 256, 56, 56) f32 -> out: (32, 256, 56, 56) f32

Layout strategy:
- activations/residuals kept compact [128, 56, 56] in SBUF (efficient DMA)
- only the binarized conv operands live in a zero/half-padded 58-wide
  layout [128, 59, 58]; the binarize op itself (is_ge -> {0,1} "u-domain",
  pads 0.5, correction folded into BN bias on host) does the spread.
- conv = 9 accumulating matmuls (one per tap) into PSUM [128, 464] per
  8-row tile; junk at cols 56/57 of each row is never consumed.
- both channel_shuffles are free: host-permuted w2 channels + stride-4
  channel DMA writes.
"""

import numpy as np
import ml_dtypes

import concourse.bass as bass
import concourse.tile as tile
from concourse import bacc, mybir
from concourse import bass_utils

EPS = 1e-5
P = 128
H = W = 56
WP = 58          # padded row width
RP = 59          # padded rows allocated (58 used + 1 spare for tail matmul reads)
IMGS_PER_CORE = 4
NCORES = 8
NT = 7           # row tiles per image (8 output rows each)
TF = 8 * WP      # matmul free size per tile = 464
CF = 8 * W       # compact free size per tile = 448

F32 = mybir.dt.float32
BF16 = mybir.dt.bfloat16
ALU = mybir.AluOpType
ACTF = mybir.ActivationFunctionType

_CACHE = {}


def _flat(ap3):
    return ap3.rearrange("p r c -> p (r c)")


def _build():
    nc = bacc.Bacc("TRN2", target_bir_lowering=False, debug=False)

    x_h = nc.dram_tensor("xs", [IMGS_PER_CORE, 2 * P, H, W], F32, kind="ExternalInput")
    w1_h = nc.dram_tensor("w1m", [P, 9 * P], BF16, kind="ExternalInput")
    w2_h = nc.dram_tensor("w2m", [P, 9 * P], BF16, kind="ExternalInput")
    cst_h = nc.dram_tensor("cst", [P, 16], F32, kind="ExternalInput")
    out_h = nc.dram_tensor("out", [IMGS_PER_CORE, 2 * P, H, W], F32, kind="ExternalOutput")

    x_ap = x_h.ap()

    def out_ch4(n, base_ch, nch=64):
        # DRAM AP: channels base_ch, base_ch+4, ... of image n, full spatial.
        return bass.AP(
            tensor=out_h,
            offset=(n * 2 * P + base_ch) * H * W,
            ap=[[4 * H * W, nch], [1, H * W]],
        )

    with tile.TileContext(nc) as tc:
        # persistent ping-pong buffers (compact except B1/B2)
        XA = [nc.alloc_sbuf_tensor(f"XA{i}", [P, H, W], F32).ap() for i in range(2)]
        A2 = [nc.alloc_sbuf_tensor(f"A2{i}", [P, H, W], F32).ap() for i in range(2)]
        B1 = nc.alloc_sbuf_tensor("B1", [P, RP, WP], BF16).ap()
        B2 = [nc.alloc_sbuf_tensor(f"B2{i}", [P, RP, WP], BF16).ap() for i in range(2)]
        XIH = [nc.alloc_sbuf_tensor(f"XIH{i}", [P, H, W], F32).ap() for i in range(2)]
        FO1 = [nc.alloc_sbuf_tensor(f"FO1{i}", [P, H, W], F32).ap() for i in range(2)]
        OT2 = [nc.alloc_sbuf_tensor(f"OT2{i}", [P, H, W], F32).ap() for i in range(2)]
        WS1 = nc.alloc_sbuf_tensor("WS1", [P, 9 * P], BF16).ap()
        WS2 = nc.alloc_sbuf_tensor("WS2", [P, 9 * P], BF16).ap()
        CST = nc.alloc_sbuf_tensor("CST", [P, 16], F32).ap()

        s1 = CST[:, 0:1]
        b1 = CST[:, 1:2]
        s2 = CST[:, 2:3]
        b2 = CST[:, 3:4]
        beta_hi = CST[64:128, 4:5]
        mv0_lo = CST[64:128, 7:8]
        cxh = CST[64:128, 8:9]

        nc.sync.dma_start(out=WS1, in_=w1_h.ap())
        nc.sync.dma_start(out=WS2, in_=w2_h.ap())
        nc.sync.dma_start(out=CST, in_=cst_h.ap())

        # u-domain pads: 0.5 stands for binarized zero-padding. Interiors
        # are rewritten per image; pads never touched again.
        nc.gpsimd.memset(_flat(B1), 0.5)
        nc.gpsimd.memset(_flat(B2[0]), 0.5)
        nc.gpsimd.memset(_flat(B2[1]), 0.5)

        with (
            tc.tile_pool(name="psum", bufs=8, space="PSUM") as psum_pool,
            tc.tile_pool(name="stage", bufs=8) as stage_pool,
        ):
            def prelude(n):
                """Loads + input binarize + idle-half bias adds for image n."""
                s = n % 2
                xa, a2, xih = XA[s], A2[s], XIH[s]
                nc.sync.dma_start(out=xa, in_=x_ap[n, 0:P])
                nc.sync.dma_start(out=a2[64:128], in_=x_ap[n, P:P + 64])
                nc.sync.dma_start(out=xih[64:128], in_=x_ap[n, P + 64:2 * P])
                # u1 = (x_act >= 0) in {0,1}, spread into padded B1
                nc.vector.tensor_scalar(
                    out=B1[:, 1:57, 1:57], in0=xa,
                    scalar1=0.0, scalar2=None, op0=ALU.is_ge)
                # idle halves: bias adds (AP scalars -> ACT/DVE only)
                nc.scalar.activation(
                    a2[64:128], a2[64:128], ACTF.Identity, bias=mv0_lo)
                nc.scalar.activation(
                    xih[64:128], xih[64:128], ACTF.Identity, bias=cxh)
                nc.sync.dma_start(out=out_ch4(n, 3), in_=_flat(xih)[64:128])

            def conv1(n):
                s = n % 2
                xa, a2, fo1 = XA[s], A2[s], FO1[s]
                b1f = _flat(B1)
                for t in range(NT):
                    ps = psum_pool.tile([P, TF], F32)
                    for k in range(9):
                        ky, kx = divmod(k, 3)
                        off = (8 * t + ky) * WP + kx
                        nc.tensor.matmul(
                            ps, lhsT=WS1[:, P * k:P * (k + 1)],
                            rhs=b1f[:, off:off + TF],
                            start=(k == 0), stop=(k == 8))
                    t1 = stage_pool.tile([P, TF], F32, tag="t1")
                    t13 = t1.rearrange("p (r c) -> p r c", c=WP)[:, :, 0:W]
                    nc.scalar.activation(t1, ps, ACTF.Identity, bias=b1, scale=s1)
                    nc.vector.tensor_tensor(
                        out=t13, in0=t13, in1=xa[:, 8 * t:8 * t + 8, :], op=ALU.add)
                    # lo channels -> A2 (conv2 input/residual)
                    nc.gpsimd.tensor_scalar(
                        out=a2[0:64, 8 * t:8 * t + 8, :], in0=t13[0:64],
                        scalar1=1.0, scalar2=-1.0, op0=ALU.min, op1=ALU.max)
                    # hi channels: clip in place, then +move1_even -> FO1
                    nc.gpsimd.tensor_scalar(
                        out=t13[64:128], in0=t13[64:128],
                        scalar1=1.0, scalar2=-1.0, op0=ALU.min, op1=ALU.max)
                    nc.vector.tensor_scalar(
                        out=fo1[64:128, 8 * t:8 * t + 8, :], in0=t13[64:128],
                        scalar1=beta_hi, scalar2=None, op0=ALU.add)
                nc.sync.dma_start(out=out_ch4(n, 1), in_=_flat(fo1)[64:128])
                # u2 = (a2 >= 0) in {0,1}, spread into padded B2
                nc.vector.tensor_scalar(
                    out=B2[s][:, 1:57, 1:57], in0=a2,
                    scalar1=0.0, scalar2=None, op0=ALU.is_ge)

            def conv2(n):
                s = n % 2
                a2, ot2 = A2[s], OT2[s]
                b2f = _flat(B2[s])
                for t in range(NT):
                    ps = psum_pool.tile([P, TF], F32)
                    for k in range(9):
                        ky, kx = divmod(k, 3)
                        off = (8 * t + ky) * WP + kx
                        nc.tensor.matmul(
                            ps, lhsT=WS2[:, P * k:P * (k + 1)],
                            rhs=b2f[:, off:off + TF],
                            start=(k == 0), stop=(k == 8))
                    t2 = stage_pool.tile([P, TF], F32, tag="t2")
                    t23 = t2.rearrange("p (r c) -> p r c", c=WP)[:, :, 0:W]
                    nc.scalar.activation(t2, ps, ACTF.Identity, bias=b2, scale=s2)
                    nc.vector.tensor_tensor(
                        out=t23, in0=t23, in1=a2[:, 8 * t:8 * t + 8, :], op=ALU.add)
                    nc.gpsimd.tensor_scalar(
                        out=ot2[:, 8 * t:8 * t + 8, :], in0=t23,
                        scalar1=1.0, scalar2=-1.0, op0=ALU.min, op1=ALU.max)
                nc.sync.dma_start(out=out_ch4(n, 0), in_=_flat(ot2)[0:64])
                nc.sync.dma_start(out=out_ch4(n, 2), in_=_flat(ot2)[64:128])

            # software pipeline across images: conv1(n+1) is emitted before
            # conv2(n) so the PE never stalls on the u2(n) dependency chain.
            prelude(0)
            conv1(0)
            for n in range(IMGS_PER_CORE):
                if n + 1 < IMGS_PER_CORE:
                    prelude(n + 1)
                    conv1(n + 1)
                conv2(n)

    nc.compile()
    return nc


def _host_prep(w1, w2, bn1_gamma, bn1_beta, bn1_mean, bn1_var,
               bn2_gamma, bn2_beta, bn2_mean, bn2_var, move0_bias, move1_bias):
    f8 = np.float64
    bw1 = np.where(w1 >= 0, 1.0, -1.0).astype(f8)   # [co, ci, 3, 3]
    bw2 = np.where(w2 >= 0, 1.0, -1.0).astype(f8)

    # conv1 lhsT layout [ci, 9*co]: col k*128+co = bw1[co, ci, ky, kx]
    w1m = np.ascontiguousarray(
        bw1.transpose(1, 2, 3, 0).reshape(P, 9 * P)).astype(ml_dtypes.bfloat16)

    # conv2 channel permutation (both in and out sides)
    pidx = np.arange(P)
    chan = np.where(pidx < 64, 2 * pidx, 2 * (pidx - 64) + 1)  # partition -> x_act2 channel
    bw2p = bw2[np.ix_(chan, chan)]                  # [co', ci', 3, 3]
    w2m = np.ascontiguousarray(
        bw2p.transpose(1, 2, 3, 0).reshape(P, 9 * P)).astype(ml_dtypes.bfloat16)

    # u-domain: conv_sign = 2*conv_u - c0, c0 = sum of signed weights
    inv1 = bn1_gamma.astype(f8) / np.sqrt(bn1_var.astype(f8) + EPS)
    c0_1 = bw1.sum(axis=(1, 2, 3))
    s1 = 2.0 * inv1
    b1 = bn1_beta.astype(f8) - bn1_mean.astype(f8) * inv1 - inv1 * c0_1

    inv2 = (bn2_gamma.astype(f8) / np.sqrt(bn2_var.astype(f8) + EPS))[chan]
    c0_2 = bw2.sum(axis=(1, 2, 3))[chan]
    s2 = 2.0 * inv2
    b2 = bn2_beta.astype(f8)[chan] - bn2_mean.astype(f8)[chan] * inv2 - inv2 * c0_2

    cst = np.zeros((P, 16), np.float64)
    cst[:, 0] = s1
    cst[:, 1] = b1
    cst[:, 2] = s2
    cst[:, 3] = b2
    i = np.arange(64)
    cst[64:128, 4] = move1_bias[2 * i]
    cst[64:128, 7] = move0_bias[i]
    cst[64:128, 8] = move0_bias[64 + i] + move1_bias[2 * i + 1]
    return w1m, w2m, cst.astype(np.float32)


def kernel(x, w1, w2, bn1_gamma, bn1_beta, bn1_mean, bn1_var,
           bn2_gamma, bn2_beta, bn2_mean, bn2_var, move0_bias, move1_bias,
           _trace=False):
    x = np.asarray(x, np.float32)
    args = [np.asarray(a, np.float32) for a in (
        w1, w2, bn1_gamma, bn1_beta, bn1_mean, bn1_var,
        bn2_gamma, bn2_beta, bn2_mean, bn2_var, move0_bias, move1_bias)]
    w1m, w2m, cst = _host_prep(*args)

    if "nc" not in _CACHE:
        _CACHE["nc"] = _build()
    nc = _CACHE["nc"]

    in_maps = [
        {"xs": np.ascontiguousarray(x[IMGS_PER_CORE * c:IMGS_PER_CORE * (c + 1)]),
         "w1m": w1m, "w2m": w2m, "cst": cst}
        for c in range(NCORES)
    ]
    kw = {}
    if _trace:
        kw = dict(trace=True, trace_kwargs={"title": "basicblock"})
    res = bass_utils.run_bass_kernel_spmd(nc, in_maps, core_ids=list(range(NCORES)), **kw)
    out = np.concatenate([res.results[c]["out"] for c in range(NCORES)], axis=0)
    if _trace:
        _CACHE["last_results"] = res
    return out


# revision 5
# speedup vs baseline: 3.3689x; 1.0950x over previous
"""Trainium2 Bass kernel for a binarized BasicBlock (2x bconv3x3 + BN +
residual hardtanh + channel shuffle), data-parallel over batch on 8 cores.

Self-contained: hardcodes shapes from the problem spec.
  x: (32, 256, 56, 56) f32 -> out: (32, 256, 56, 56) f32

Layout strategy:
- activations/residuals kept compact [128, 56, 56] in SBUF (efficient DMA)
- only the binarized conv operands live in a zero/half-padded 58-wide
  layout [128, 59, 58]; the binarize op itself (is_ge -> {0,1} "u-domain",
  pads 0.5, correction folded into BN bias on host) does the spread.
- conv = 9 accumulating matmuls (one per tap) into PSUM [128, 464] per
  8-row tile; junk at cols 56/57 of each row is never consumed.
- both channel_shuffles are free: host-permuted w2 channels + stride-4
  channel DMA writes.
"""

import numpy as np
import ml_dtypes

import concourse.bass as bass
import concourse.tile as tile
from concourse import bacc, mybir
from concourse import bass_utils

EPS = 1e-5
P = 128
H = W = 56
WP = 58          # padded row width
RP = 59          # padded rows allocated (58 used + 1 spare for tail matmul reads)
IMGS_PER_CORE = 4
NCORES = 8
NT = 7           # row tiles per image (8 output rows each)
TF = 8 * WP      # matmul free size per tile = 464
CF = 8 * W       # compact free size per tile = 448

F32 = mybir.dt.float32
BF16 = mybir.dt.bfloat16
ALU = mybir.AluOpType
ACTF = mybir.ActivationFunctionType

_CACHE = {}


def _flat(ap3):
    return ap3.rearrange("p r c -> p (r c)")


def _build():
    nc = bacc.Bacc("TRN2", target_bir_lowering=False, debug=False)

    x_h = nc.dram_tensor("xs", [IMGS_PER_CORE, 2 * P, H, W], F32, kind="ExternalInput")
    w1_h = nc.dram_tensor("w1m", [P, 9 * P], BF16, kind="ExternalInput")
    w2_h = nc.dram_tensor("w2m", [P, 9 * P], BF16, kind="ExternalInput")
    cst_h = nc.dram_tensor("cst", [P, 16], F32, kind="ExternalInput")
    out_h = nc.dram_tensor("out", [IMGS_PER_CORE, 2 * P, H, W], F32, kind="ExternalOutput")

    x_ap = x_h.ap()

    def out_ch4(n, base_ch, nch=64):
        # DRAM AP: channels base_ch, base_ch+4, ... of image n, full spatial.
        return bass.AP(
            tensor=out_h,
            offset=(n * 2 * P + base_ch) * H * W,
            ap=[[4 * H * W, nch], [1, H * W]],
        )

    with tile.TileContext(nc) as tc:
        # persistent ping-pong buffers (compact except B1/B2)
        XA = [nc.alloc_sbuf_tensor(f"XA{i}", [P, H, W], F32).ap() for i in range(2)]
        A2 = [nc.alloc_sbuf_tensor(f"A2{i}", [P, H, W], F32).ap() for i in range(2)]
        B1 = nc.alloc_sbuf_tensor("B1", [P, RP, WP], BF16).ap()
        B2 = [nc.alloc_sbuf_tensor(f"B2{i}", [P, RP, WP], BF16).ap() for i in range(2)]
        XIH = [nc.alloc_sbuf_tensor(f"XIH{i}", [P, H, W], F32).ap() for i in range(2)]
        FO1 = [nc.alloc_sbuf_tensor(f"FO1{i}", [P, H, W], F32).ap() for i in range(2)]
        OT2 = [nc.alloc_sbuf_tensor(f"OT2{i}", [P, H, W], F32).ap() for i in range(2)]
        WS1 = nc.alloc_sbuf_tensor("WS1", [P, 9 * P], BF16).ap()
        WS2 = nc.alloc_sbuf_tensor("WS2", [P, 9 * P], BF16).ap()
        CST = nc.alloc_sbuf_tensor("CST", [P, 16], F32).ap()

        s1 = CST[:, 0:1]
        b1 = CST[:, 1:2]
        s2 = CST[:, 2:3]
        b2 = CST[:, 3:4]
        beta_hi = CST[64:128, 4:5]
        mv0_lo = CST[64:128, 7:8]
        cxh = CST[64:128, 8:9]

        nc.sync.dma_start(out=WS1, in_=w1_h.ap())
        nc.sync.dma_start(out=WS2, in_=w2_h.ap())
        nc.sync.dma_start(out=CST, in_=cst_h.ap())

        # u-domain pads: 0.5 stands for binarized zero-padding. Interiors
        # are rewritten per image; pads never touched again.
        nc.gpsimd.memset(_flat(B1), 0.5)
        nc.gpsimd.memset(_flat(B2[0]), 0.5)
        nc.gpsimd.memset(_flat(B2[1]), 0.5)

        with (
            tc.tile_pool(name="psum", bufs=8, space="PSUM") as psum_pool,
            tc.tile_pool(name="stage", bufs=8) as stage_pool,
        ):
            def prelude(n):
                """Loads + input binarize + idle-half bias adds for image n."""
                s = n % 2
                xa, a2, xih = XA[s], A2[s], XIH[s]
                nc.sync.dma_start(out=xa, in_=x_ap[n, 0:P])
                nc.sync.dma_start(out=a2[64:128], in_=x_ap[n, P:P + 64])
                nc.sync.dma_start(out=xih[64:128], in_=x_ap[n, P + 64:2 * P])
                # u1 = (x_act >= 0) in {0,1}, spread into padded B1
                nc.vector.tensor_scalar(
                    out=B1[:, 1:57, 1:57], in0=xa,
                    scalar1=0.0, scalar2=None, op0=ALU.is_ge)
                # idle halves: bias adds (AP scalars -> ACT/DVE only)
                nc.scalar.activation(
                    a2[64:128], a2[64:128], ACTF.Identity, bias=mv0_lo)
                nc.scalar.activation(
                    xih[64:128], xih[64:128], ACTF.Identity, bias=cxh)
                nc.scalar.dma_start(out=out_ch4(n, 3), in_=_flat(xih)[64:128])

            def conv1(n):
                s = n % 2
                xa, a2, fo1 = XA[s], A2[s], FO1[s]
                b1f = _flat(B1)
                for t in range(NT):
                    ps = psum_pool.tile([P, TF], F32)
                    for k in range(9):
                        ky, kx = divmod(k, 3)
                        off = (8 * t + ky) * WP + kx
                        nc.tensor.matmul(
                            ps, lhsT=WS1[:, P * k:P * (k + 1)],
                            rhs=b1f[:, off:off + TF],
                            start=(k == 0), stop=(k == 8))
                    t1 = stage_pool.tile([P, TF], F32, tag="t1")
                    t13 = t1.rearrange("p (r c) -> p r c", c=WP)[:, :, 0:W]
                    nc.scalar.activation(t1, ps, ACTF.Identity, bias=b1, scale=s1)
                    nc.vector.tensor_tensor(
                        out=t13, in0=t13, in1=xa[:, 8 * t:8 * t + 8, :], op=ALU.add)
                    # lo channels -> A2 (conv2 input/residual)
                    nc.gpsimd.tensor_scalar(
                        out=a2[0:64, 8 * t:8 * t + 8, :], in0=t13[0:64],
                        scalar1=1.0, scalar2=-1.0, op0=ALU.min, op1=ALU.max)
                    # hi channels: clip in place, then +move1_even -> FO1
                    nc.gpsimd.tensor_scalar(
                        out=t13[64:128], in0=t13[64:128],
                        scalar1=1.0, scalar2=-1.0, op0=ALU.min, op1=ALU.max)
                    nc.vector.tensor_scalar(
                        out=fo1[64:128, 8 * t:8 * t + 8, :], in0=t13[64:128],
                        scalar1=beta_hi, scalar2=None, op0=ALU.add)
                nc.scalar.dma_start(out=out_ch4(n, 1), in_=_flat(fo1)[64:128])
                # u2 = (a2 >= 0) in {0,1}, spread into padded B2
                nc.vector.tensor_scalar(
                    out=B2[s][:, 1:57, 1:57], in0=a2,
                    scalar1=0.0, scalar2=None, op0=ALU.is_ge)

            def conv2(n):
                s = n % 2
                a2, ot2 = A2[s], OT2[s]
                b2f = _flat(B2[s])
                for t in range(NT):
                    ps = psum_pool.tile([P, TF], F32)
                    for k in range(9):
                        ky, kx = divmod(k, 3)
                        off = (8 * t + ky) * WP + kx
                        nc.tensor.matmul(
                            ps, lhsT=WS2[:, P * k:P * (k + 1)],
                            rhs=b2f[:, off:off + TF],
                            start=(k == 0), stop=(k == 8))
                    t2 = stage_pool.tile([P, TF], F32, tag="t2")
                    t23 = t2.rearrange("p (r c) -> p r c", c=WP)[:, :, 0:W]
                    nc.scalar.activation(t2, ps, ACTF.Identity, bias=b2, scale=s2)
                    nc.vector.tensor_tensor(
                        out=t23, in0=t23, in1=a2[:, 8 * t:8 * t + 8, :], op=ALU.add)
                    nc.gpsimd.tensor_scalar(
                        out=ot2[:, 8 * t:8 * t + 8, :], in0=t23,
                        scalar1=1.0, scalar2=-1.0, op0=ALU.min, op1=ALU.max)
                nc.scalar.dma_start(out=out_ch4(n, 0), in_=_flat(ot2)[0:64])
                nc.scalar.dma_start(out=out_ch4(n, 2), in_=_flat(ot2)[64:128])

            # software pipeline across images: conv1(n+1) is emitted before
            # conv2(n) so the PE never stalls on the u2(n) dependency chain.
            prelude(0)
            conv1(0)
            for n in range(IMGS_PER_CORE):
                if n + 1 < IMGS_PER_CORE:
                    prelude(n + 1)
                    conv1(n + 1)
                conv2(n)

    nc.compile()
    return nc


def _host_prep(w1, w2, bn1_gamma, bn1_beta, bn1_mean, bn1_var,
               bn2_gamma, bn2_beta, bn2_mean, bn2_var, move0_bias, move1_bias):
    f8 = np.float64
    bw1 = np.where(w1 >= 0, 1.0, -1.0).astype(f8)   # [co, ci, 3, 3]
    bw2 = np.where(w2 >= 0, 1.0, -1.0).astype(f8)

    # conv1 lhsT layout [ci, 9*co]: col k*128+co = bw1[co, ci, ky, kx]
    w1m = np.ascontiguousarray(
        bw1.transpose(1, 2, 3, 0).reshape(P, 9 * P)).astype(ml_dtypes.bfloat16)

    # conv2 channel permutation (both in and out sides)
    pidx = np.arange(P)
    chan = np.where(pidx < 64, 2 * pidx, 2 * (pidx - 64) + 1)  # partition -> x_act2 channel
    bw2p = bw2[np.ix_(chan, chan)]                  # [co', ci', 3, 3]
    w2m = np.ascontiguousarray(
        bw2p.transpose(1, 2, 3, 0).reshape(P, 9 * P)).astype(ml_dtypes.bfloat16)

    # u-domain: conv_sign = 2*conv_u - c0, c0 = sum of signed weights
    inv1 = bn1_gamma.astype(f8) / np.sqrt(bn1_var.astype(f8) + EPS)
    c0_1 = bw1.sum(axis=(1, 2, 3))
    s1 = 2.0 * inv1
    b1 = bn1_beta.astype(f8) - bn1_mean.astype(f8) * inv1 - inv1 * c0_1

    inv2 = (bn2_gamma.astype(f8) / np.sqrt(bn2_var.astype(f8) + EPS))[chan]
    c0_2 = bw2.sum(axis=(1, 2, 3))[chan]
    s2 = 2.0 * inv2
    b2 = bn2_beta.astype(f8)[chan] - bn2_mean.astype(f8)[chan] * inv2 - inv2 * c0_2

    cst = np.zeros((P, 16), np.float64)
    cst[:, 0] = s1
    cst[:, 1] = b1
    cst[:, 2] = s2
    cst[:, 3] = b2
    i = np.arange(64)
    cst[64:128, 4] = move1_bias[2 * i]
    cst[64:128, 7] = move0_bias[i]
    cst[64:128, 8] = move0_bias[64 + i] + move1_bias[2 * i + 1]
    return w1m, w2m, cst.astype(np.float32)


def kernel(x, w1, w2, bn1_gamma, bn1_beta, bn1_mean, bn1_var,
           bn2_gamma, bn2_beta, bn2_mean, bn2_var, move0_bias, move1_bias,
           _trace=False):
    x = np.asarray(x, np.float32)
    args = [np.asarray(a, np.float32) for a in (
        w1, w2, bn1_gamma, bn1_beta, bn1_mean, bn1_var,
        bn2_gamma, bn2_beta, bn2_mean, bn2_var, move0_bias, move1_bias)]
    w1m, w2m, cst = _host_prep(*args)

    if "nc" not in _CACHE:
        _CACHE["nc"] = _build()
    nc = _CACHE["nc"]

    in_maps = [
        {"xs": np.ascontiguousarray(x[IMGS_PER_CORE * c:IMGS_PER_CORE * (c + 1)]),
         "w1m": w1m, "w2m": w2m, "cst": cst}
        for c in range(NCORES)
    ]
    kw = {}
    if _trace:
        kw = dict(trace=True, trace_kwargs={"title": "basicblock"})
    res = bass_utils.run_bass_kernel_spmd(nc, in_maps, core_ids=list(range(NCORES)), **kw)
    out = np.concatenate([res.results[c]["out"] for c in range(NCORES)], axis=0)
    if _trace:
        _CACHE["last_results"] = res
    return out


# revision 6
# speedup vs baseline: 3.4948x; 1.0374x over previous
"""Trainium2 Bass kernel for a binarized BasicBlock (2x bconv3x3 + BN +
residual hardtanh + channel shuffle), data-parallel over batch on 8 cores.

Self-contained: hardcodes shapes from the problem spec.
  x: (32, 256, 56, 56) f32 -> out: (32, 256, 56, 56) f32

Layout strategy:
- activations/residuals kept compact [128, 56, 56] in SBUF (efficient DMA)
- only the binarized conv operands live in a zero/half-padded 58-wide
  layout [128, 59, 58]; the binarize op itself (is_ge -> {0,1} "u-domain",
  pads 0.5, correction folded into BN bias on host) does the spread.
- conv = 9 accumulating matmuls (one per tap) into PSUM [128, 464] per
  8-row tile; junk at cols 56/57 of each row is never consumed.
- both channel_shuffles are free: host-permuted w2 channels + stride-4
  channel DMA writes.
"""

import numpy as np
import ml_dtypes

import concourse.bass as bass
import concourse.tile as tile
from concourse import bacc, mybir
from concourse import bass_utils

EPS = 1e-5
P = 128
H = W = 56
WP = 58          # padded row width
RP = 59          # padded rows allocated (58 used + 1 spare for tail matmul reads)
IMGS_PER_CORE = 4
NCORES = 8
NT = 7           # row tiles per image (8 output rows each)
TF = 8 * WP      # matmul free size per tile = 464
CF = 8 * W       # compact free size per tile = 448

F32 = mybir.dt.float32
BF16 = mybir.dt.bfloat16
ALU = mybir.AluOpType
ACTF = mybir.ActivationFunctionType

_CACHE = {}


def _flat(ap3):
    return ap3.rearrange("p r c -> p (r c)")


def _build():
    nc = bacc.Bacc("TRN2", target_bir_lowering=False, debug=False)

    x_h = nc.dram_tensor("xs", [IMGS_PER_CORE, 2 * P, H, W], F32, kind="ExternalInput")
    w1_h = nc.dram_tensor("w1m", [P, 9 * P], BF16, kind="ExternalInput")
    w2_h = nc.dram_tensor("w2m", [P, 9 * P], BF16, kind="ExternalInput")
    cst_h = nc.dram_tensor("cst", [P, 16], F32, kind="ExternalInput")
    out_h = nc.dram_tensor("out", [IMGS_PER_CORE, 2 * P, H, W], F32, kind="ExternalOutput")

    x_ap = x_h.ap()

    def out_ch4(n, base_ch, nch=64, half=None):
        # DRAM AP: channels base_ch, base_ch+4, ... of image n, full spatial
        # (or the first/second 32-row half when half is 0/1).
        off, sz = 0, H * W
        if half is not None:
            off = half * 32 * W
            sz = 32 * W if half == 0 else (H - 32) * W
        return bass.AP(
            tensor=out_h,
            offset=(n * 2 * P + base_ch) * H * W + off,
            ap=[[4 * H * W, nch], [1, sz]],
        )

    with tile.TileContext(nc) as tc:
        # persistent ping-pong buffers (compact except B1/B2)
        XA = [nc.alloc_sbuf_tensor(f"XA{i}", [P, H, W], F32).ap() for i in range(2)]
        A2 = [nc.alloc_sbuf_tensor(f"A2{i}", [P, H, W], F32).ap() for i in range(2)]
        B1 = nc.alloc_sbuf_tensor("B1", [P, RP, WP], BF16).ap()
        B2 = [nc.alloc_sbuf_tensor(f"B2{i}", [P, RP, WP], BF16).ap() for i in range(2)]
        XIH = [nc.alloc_sbuf_tensor(f"XIH{i}", [P, H, W], F32).ap() for i in range(2)]
        FO1 = [nc.alloc_sbuf_tensor(f"FO1{i}", [P, H, W], F32).ap() for i in range(2)]
        OT2 = [nc.alloc_sbuf_tensor(f"OT2{i}", [P, H, W], F32).ap() for i in range(2)]
        WS1 = nc.alloc_sbuf_tensor("WS1", [P, 9 * P], BF16).ap()
        WS2 = nc.alloc_sbuf_tensor("WS2", [P, 9 * P], BF16).ap()
        CST = nc.alloc_sbuf_tensor("CST", [P, 16], F32).ap()

        s1 = CST[:, 0:1]
        b1 = CST[:, 1:2]
        s2 = CST[:, 2:3]
        b2 = CST[:, 3:4]
        beta_hi = CST[64:128, 4:5]
        mv0_lo = CST[64:128, 7:8]
        cxh = CST[64:128, 8:9]

        nc.sync.dma_start(out=WS1, in_=w1_h.ap())
        nc.sync.dma_start(out=WS2, in_=w2_h.ap())
        nc.sync.dma_start(out=CST, in_=cst_h.ap())

        # u-domain pads: 0.5 stands for binarized zero-padding. Interiors
        # are rewritten per image; pads never touched again.
        nc.gpsimd.memset(_flat(B1), 0.5)
        nc.gpsimd.memset(_flat(B2[0]), 0.5)
        nc.gpsimd.memset(_flat(B2[1]), 0.5)

        with (
            tc.tile_pool(name="psum", bufs=8, space="PSUM") as psum_pool,
            tc.tile_pool(name="stage", bufs=8) as stage_pool,
        ):
            def prelude(n):
                """Loads + input binarize + idle-half bias adds for image n."""
                s = n % 2
                xa, a2, xih = XA[s], A2[s], XIH[s]
                if n == 0:
                    # head: chunk the first load so conv1(0) starts sooner
                    nc.sync.dma_start(out=xa[:, 0:32], in_=x_ap[n, 0:P, 0:32])
                    nc.sync.dma_start(out=xa[:, 32:56], in_=x_ap[n, 0:P, 32:56])
                else:
                    nc.sync.dma_start(out=xa, in_=x_ap[n, 0:P])
                nc.sync.dma_start(out=a2[64:128], in_=x_ap[n, P:P + 64])
                nc.sync.dma_start(out=xih[64:128], in_=x_ap[n, P + 64:2 * P])
                # u1 = (x_act >= 0) in {0,1}, spread into padded B1 (2 chunks)
                nc.vector.tensor_scalar(
                    out=B1[:, 1:33, 1:57], in0=xa[:, 0:32],
                    scalar1=0.0, scalar2=None, op0=ALU.is_ge)
                nc.vector.tensor_scalar(
                    out=B1[:, 33:57, 1:57], in0=xa[:, 32:56],
                    scalar1=0.0, scalar2=None, op0=ALU.is_ge)
                # idle halves: bias adds (AP scalars -> ACT/DVE only)
                nc.scalar.activation(
                    a2[64:128], a2[64:128], ACTF.Identity, bias=mv0_lo)
                nc.scalar.activation(
                    xih[64:128], xih[64:128], ACTF.Identity, bias=cxh)
                nc.scalar.dma_start(out=out_ch4(n, 3), in_=_flat(xih)[64:128])

            def conv1(n):
                s = n % 2
                xa, a2, fo1 = XA[s], A2[s], FO1[s]
                b1f = _flat(B1)
                for t in range(NT):
                    ps = psum_pool.tile([P, TF], F32)
                    for k in range(9):
                        ky, kx = divmod(k, 3)
                        off = (8 * t + ky) * WP + kx
                        nc.tensor.matmul(
                            ps, lhsT=WS1[:, P * k:P * (k + 1)],
                            rhs=b1f[:, off:off + TF],
                            start=(k == 0), stop=(k == 8))
                    t1 = stage_pool.tile([P, TF], F32, tag="t1")
                    t13 = t1.rearrange("p (r c) -> p r c", c=WP)[:, :, 0:W]
                    nc.scalar.activation(t1, ps, ACTF.Identity, bias=b1, scale=s1)
                    nc.vector.tensor_tensor(
                        out=t13, in0=t13, in1=xa[:, 8 * t:8 * t + 8, :], op=ALU.add)
                    # lo channels -> A2 (conv2 input/residual)
                    nc.gpsimd.tensor_scalar(
                        out=a2[0:64, 8 * t:8 * t + 8, :], in0=t13[0:64],
                        scalar1=1.0, scalar2=-1.0, op0=ALU.min, op1=ALU.max)
                    # hi channels: clip in place, then +move1_even -> FO1
                    nc.gpsimd.tensor_scalar(
                        out=t13[64:128], in0=t13[64:128],
                        scalar1=1.0, scalar2=-1.0, op0=ALU.min, op1=ALU.max)
                    nc.vector.tensor_scalar(
                        out=fo1[64:128, 8 * t:8 * t + 8, :], in0=t13[64:128],
                        scalar1=beta_hi, scalar2=None, op0=ALU.add)
                nc.scalar.dma_start(out=out_ch4(n, 1), in_=_flat(fo1)[64:128])
                # u2 = (a2 >= 0) in {0,1}, spread into padded B2 (2 chunks)
                nc.vector.tensor_scalar(
                    out=B2[s][:, 1:33, 1:57], in0=a2[:, 0:32],
                    scalar1=0.0, scalar2=None, op0=ALU.is_ge)
                nc.vector.tensor_scalar(
                    out=B2[s][:, 33:57, 1:57], in0=a2[:, 32:56],
                    scalar1=0.0, scalar2=None, op0=ALU.is_ge)

            def conv2(n):
                s = n % 2
                a2, ot2 = A2[s], OT2[s]
                b2f = _flat(B2[s])
                final = n == IMGS_PER_CORE - 1
                for t in range(NT):
                    ps = psum_pool.tile([P, TF], F32)
                    for k in range(9):
                        ky, kx = divmod(k, 3)
                        off = (8 * t + ky) * WP + kx
                        nc.tensor.matmul(
                            ps, lhsT=WS2[:, P * k:P * (k + 1)],
                            rhs=b2f[:, off:off + TF],
                            start=(k == 0), stop=(k == 8))
                    t2 = stage_pool.tile([P, TF], F32, tag="t2")
                    t23 = t2.rearrange("p (r c) -> p r c", c=WP)[:, :, 0:W]
                    nc.scalar.activation(t2, ps, ACTF.Identity, bias=b2, scale=s2)
                    nc.vector.tensor_tensor(
                        out=t23, in0=t23, in1=a2[:, 8 * t:8 * t + 8, :], op=ALU.add)
                    nc.gpsimd.tensor_scalar(
                        out=ot2[:, 8 * t:8 * t + 8, :], in0=t23,
                        scalar1=1.0, scalar2=-1.0, op0=ALU.min, op1=ALU.max)
                    if final and t == 3:
                        # tail: flush the first half as soon as it's ready
                        h = 32 * W
                        nc.scalar.dma_start(
                            out=out_ch4(n, 0, half=0), in_=_flat(ot2)[0:64, 0:h])
                        nc.scalar.dma_start(
                            out=out_ch4(n, 2, half=0), in_=_flat(ot2)[64:128, 0:h])
                if final:
                    h = 32 * W
                    nc.scalar.dma_start(
                        out=out_ch4(n, 0, half=1), in_=_flat(ot2)[0:64, h:H * W])
                    nc.scalar.dma_start(
                        out=out_ch4(n, 2, half=1), in_=_flat(ot2)[64:128, h:H * W])
                else:
                    nc.scalar.dma_start(out=out_ch4(n, 0), in_=_flat(ot2)[0:64])
                    nc.scalar.dma_start(out=out_ch4(n, 2), in_=_flat(ot2)[64:128])

            # software pipeline across images: conv1(n+1) is emitted before
            # conv2(n) so the PE never stalls on the u2(n) dependency chain.
            prelude(0)
            conv1(0)
            for n in range(IMGS_PER_CORE):
                if n + 1 < IMGS_PER_CORE:
                    prelude(n + 1)
                    conv1(n + 1)
                conv2(n)

    nc.compile()
    return nc


def _host_prep(w1, w2, bn1_gamma, bn1_beta, bn1_mean, bn1_var,
               bn2_gamma, bn2_beta, bn2_mean, bn2_var, move0_bias, move1_bias):
    f8 = np.float64
    bw1 = np.where(w1 >= 0, 1.0, -1.0).astype(f8)   # [co, ci, 3, 3]
    bw2 = np.where(w2 >= 0, 1.0, -1.0).astype(f8)

    # conv1 lhsT layout [ci, 9*co]: col k*128+co = bw1[co, ci, ky, kx]
    w1m = np.ascontiguousarray(
        bw1.transpose(1, 2, 3, 0).reshape(P, 9 * P)).astype(ml_dtypes.bfloat16)

    # conv2 channel permutation (both in and out sides)
    pidx = np.arange(P)
    chan = np.where(pidx < 64, 2 * pidx, 2 * (pidx - 64) + 1)  # partition -> x_act2 channel
    bw2p = bw2[np.ix_(chan, chan)]                  # [co', ci', 3, 3]
    w2m = np.ascontiguousarray(
        bw2p.transpose(1, 2, 3, 0).reshape(P, 9 * P)).astype(ml_dtypes.bfloat16)

    # u-domain: conv_sign = 2*conv_u - c0, c0 = sum of signed weights
    inv1 = bn1_gamma.astype(f8) / np.sqrt(bn1_var.astype(f8) + EPS)
    c0_1 = bw1.sum(axis=(1, 2, 3))
    s1 = 2.0 * inv1
    b1 = bn1_beta.astype(f8) - bn1_mean.astype(f8) * inv1 - inv1 * c0_1

    inv2 = (bn2_gamma.astype(f8) / np.sqrt(bn2_var.astype(f8) + EPS))[chan]
    c0_2 = bw2.sum(axis=(1, 2, 3))[chan]
    s2 = 2.0 * inv2
    b2 = bn2_beta.astype(f8)[chan] - bn2_mean.astype(f8)[chan] * inv2 - inv2 * c0_2

    cst = np.zeros((P, 16), np.float64)
    cst[:, 0] = s1
    cst[:, 1] = b1
    cst[:, 2] = s2
    cst[:, 3] = b2
    i = np.arange(64)
    cst[64:128, 4] = move1_bias[2 * i]
    cst[64:128, 7] = move0_bias[i]
    cst[64:128, 8] = move0_bias[64 + i] + move1_bias[2 * i + 1]
    return w1m, w2m, cst.astype(np.float32)


def kernel(x, w1, w2, bn1_gamma, bn1_beta, bn1_mean, bn1_var,
           bn2_gamma, bn2_beta, bn2_mean, bn2_var, move0_bias, move1_bias,
           _trace=False):
    x = np.asarray(x, np.float32)
    args = [np.asarray(a, np.float32) for a in (
        w1, w2, bn1_gamma, bn1_beta, bn1_mean, bn1_var,
        bn2_gamma, bn2_beta, bn2_mean, bn2_var, move0_bias, move1_bias)]
    w1m, w2m, cst = _host_prep(*args)

    if "nc" not in _CACHE:
        _CACHE["nc"] = _build()
    nc = _CACHE["nc"]

    in_maps = [
        {"xs": np.ascontiguousarray(x[IMGS_PER_CORE * c:IMGS_PER_CORE * (c + 1)]),
         "w1m": w1m, "w2m": w2m, "cst": cst}
        for c in range(NCORES)
    ]
    kw = {}
    if _trace:
        kw = dict(trace=True, trace_kwargs={"title": "basicblock"})
    res = bass_utils.run_bass_kernel_spmd(nc, in_maps, core_ids=list(range(NCORES)), **kw)
    out = np.concatenate([res.results[c]["out"] for c in range(NCORES)], axis=0)
    if _trace:
        _CACHE["last_results"] = res
    return out
